# revision 18
# baseline (speedup 1.0000x reference)
"""Trainium2 Bass kernel for the EnrichClassifier pathway MLP.

Network (eval mode, BN folded into weights):
  h1 = relu(x @ (w1*m1).T * s1 + b1')   [8192,5000] -> [8192,4000]
  h2 = relu(h1 @ (w2*m2).T * s2 + b2')                 -> [8192,2000]
  h3 = relu(h2 @ (w3*m3).T * s3 + b3')                 -> [8192,1000]
  sc = relu(h3 @ (w4*m4).T + b4)                       -> [8192,200]
  out = sc @ wc.T + bc                                 -> [8192,50]

Structure: m1 gives each of 200 pathways a private set of 100 genes;
20 L1 units per pathway share that set. m2/m3/m4 are block-diagonal
(20->10->5->1 per pathway). Per pathway we use its pre-gathered 100
gene rows of x^T (host-packed fp8) and run tiny dense per-pathway
matmuls packed into PE column tiles.

Pathways are relabeled into PROCESSING order (ORDER): supergroups are
visited interleaving streamed/resident so gather DMA hides behind
compute, and all downstream tiles pack pathways contiguously in that
order (the classifier absorbs the permutation).

Layer tiling (per 512-column batch half), i = processing quad index:
  h1 tile i: 4 paths x 32 rows    (L1: [100,32] stationaries, 4x col strips)
  h2 tile m: 8 paths x 16 rows    (L2: [128,64], strip parity pairs)
  h3 tile n: 16 paths x 8 rows    (L3: [128,64], strip parity pairs)
  sc tiles:  T=0 paths ORDER[0:128], T=1 ORDER[128:200], 1 row/path
             (L4: [*,32] into region-sliced psum pairs)

Sharding: data parallel over batch across 8 cores (1024 rows/core);
weights replicated. NRES supergroups of xg stay resident in SBUF.
"""

import contextlib

import numpy as np

import concourse.bass as bass
import concourse.bacc as bacc
import concourse.tile as tile
import concourse.mybir as mybir
from concourse.bass_utils import run_bass_kernel_spmd

# ---------------- hardcoded geometry ----------------
B, G, NPATH = 8192, 5000, 200
NCORES = 8
BC = B // NCORES            # 1024 rows per core
NB = 512                    # PSUM bank free size (fp32) = batch half
U1, U2, U3 = 20, 10, 5      # per-pathway units per layer
NL = 50                     # labels
KPAD = 100                  # gene slots per pathway (exact, no padding)
SGS = 12                    # pathways per supergroup
NSG = 17                    # supergroups (16 full + 1 of 8)
NRES = 9                    # supergroups resident in SBUF (loaded once)
NQUAD = 50                  # h1 tiles (4 pathways each)
NH2T = 25                   # h2 tiles (8 pathways each)
NH3T = 13                   # h3 tiles (16 pathways each, last 8)
F32 = mybir.dt.float32
F16 = mybir.dt.float16
BF16 = mybir.dt.bfloat16
F8 = mybir.dt.float8e4
RELU = mybir.ActivationFunctionType.Relu
IDENT = mybir.ActivationFunctionType.Identity
ADD = mybir.AluOpType.add
MAX = mybir.AluOpType.max

_COMPILED = None  # cached nc across calls


def _sg_paths(sg):
    return range(SGS * sg, min(SGS * sg + SGS, NPATH))


def _sg_order():
    """Supergroup processing order: streamed first, interleaved with
    resident so the gather DMA always has compute to hide behind."""
    order = []
    si, ri = NRES, 0
    while si < NSG or ri < NRES:
        if si < NSG:
            order.append(si)
            si += 1
        if ri < NRES:
            order.append(ri)
            ri += 1
    return order


def _path_order():
    """Pathways in processing order (ORDER[4*i + j] = path j of quad i)."""
    return [p for sg in _sg_order() for p in _sg_paths(sg)]


def _pack(inputs):
    """Host-side packing: BN folding, per-pathway weight blocks in
    processing order, per-core gathered-gene fp8 slices of x^T."""
    import ml_dtypes

    f = lambda k: np.asarray(inputs[k], np.float32)
    x = f("x")
    w1, b1, m1 = f("w1"), f("b1"), f("m1")
    w2, b2, m2 = f("w2"), f("b2"), f("m2")
    w3, b3, m3 = f("w3"), f("b3"), f("m3")
    w4, b4, m4 = f("w4"), f("b4"), f("m4")
    wc, bc = f("wc"), f("bc")

    def fold(gamma, beta, rm, rv):
        s = gamma / np.sqrt(rv + 1e-5)
        return s, beta - rm * s

    s1, t1 = fold(f("gamma1"), f("beta1"), f("rm1"), f("rv1"))
    s2, t2 = fold(f("gamma2"), f("beta2"), f("rm2"), f("rv2"))
    s3, t3 = fold(f("gamma3"), f("beta3"), f("rm3"), f("rv3"))
    w1m = w1 * m1 * s1[:, None]
    b1f = b1 * s1 + t1
    w2m = w2 * m2 * s2[:, None]
    b2f = b2 * s2 + t2
    w3m = w3 * m3 * s3[:, None]
    b3f = b3 * s3 + t3
    w4m = w4 * m4

    ORDER = _path_order()

    # gather index table: pathway p -> its gene rows (exactly KPAD genes)
    genes = []
    idx_mat = np.zeros((NPATH, KPAD), np.int64)
    for p in range(NPATH):
        g = np.nonzero(m1[U1 * p] != 0)[0]
        assert len(g) == KPAD
        genes.append(g)
        idx_mat[p] = g

    # L1 stationary [KPAD, 32*NPATH], indexed by GLOBAL path p:
    # col 32p+u = unit u of pathway p, row k = k-th gathered gene of p
    w1s = np.zeros((KPAD, 32 * NPATH), np.float16)
    b1v = np.zeros((128, NQUAD), np.float32)
    for p in range(NPATH):
        g = genes[p]
        w1s[:, 32 * p : 32 * p + U1] = w1m[U1 * p : U1 * p + U1, g].T.astype(np.float16)
    for i in range(NQUAD):          # processing quad index
        for j in range(4):
            p = ORDER[4 * i + j]
            b1v[32 * j : 32 * j + U1, i] = b1f[U1 * p : U1 * p + U1]

    # L2 stationary per processing quad i: [128, 64]: rows 32j+u (h1 unit
    # u of ORDER[4i+j]), cols 16j+v (h2 unit v). Quad i -> h2 tile i//2,
    # rows 64*(i%2) + 16j + v.
    w2q = np.zeros((128, 64 * NQUAD), np.float32)
    for i in range(NQUAD):
        for j in range(4):
            p = ORDER[4 * i + j]
            blk = w2m[U2 * p : U2 * p + U2, U1 * p : U1 * p + U1]  # [10,20]
            w2q[32 * j : 32 * j + U1, 64 * i + 16 * j : 64 * i + 16 * j + U2] = blk.T
    b2v = np.zeros((128, NH2T), np.float32)
    for m in range(NH2T):
        for q in range(8):
            p = ORDER[8 * m + q]
            b2v[16 * q : 16 * q + U2, m] = b2f[U2 * p : U2 * p + U2]

    # L3 stationary per h2 tile m: [128, 64]: rows 16q+v, cols 8q+w.
    # h2 tile m -> h3 tile m//2, rows 64*(m%2) + 8q + w.
    w3q = np.zeros((128, 64 * NH2T), np.float32)
    for m in range(NH2T):
        for q in range(8):
            p = ORDER[8 * m + q]
            blk = w3m[U3 * p : U3 * p + U3, U2 * p : U2 * p + U2]  # [5,10]
            w3q[16 * q : 16 * q + U2, 64 * m + 8 * q : 64 * m + 8 * q + U3] = blk.T
    b3v = np.zeros((128, NH3T), np.float32)
    for n in range(NH3T):
        for r in range(min(16, NPATH - 16 * n)):
            p = ORDER[16 * n + r]
            b3v[8 * r : 8 * r + U3, n] = b3f[U3 * p : U3 * p + U3]

    # L4 stationary per h3 tile n: [128, 32]: rows 8r+w, col 16*(n%2)+r.
    # sc tile T=(0 if n<8 else 1), psum region rows 32*mp%4 + 16*(n%2)+r.
    w4q = np.zeros((128, 32 * NH3T), np.float32)
    for n in range(NH3T):
        for r in range(min(16, NPATH - 16 * n)):
            p = ORDER[16 * n + r]
            w4q[8 * r : 8 * r + U3, 32 * n + 16 * (n % 2) + r] = w4m[p, U3 * p : U3 * p + U3]
    b4v = np.zeros((128, 2), np.float32)
    b4v[:, 0] = b4[np.asarray(ORDER[:128])]
    b4v[:72, 1] = b4[np.asarray(ORDER[128:])]

    # classifier: sc tile A rows = ORDER[0:128], B rows = ORDER[128:200].
    # cols 0-49 labels for half0 (psum rows 0-63), cols 64-113 for half1.
    wcs = np.zeros((128, 256), np.float32)
    wcs[:, 0:NL] = wc[:, np.asarray(ORDER[:128])].T
    wcs[:, 64 : 64 + NL] = wc[:, np.asarray(ORDER[:128])].T
    wcs[:72, 128 : 128 + NL] = wc[:, np.asarray(ORDER[128:])].T
    wcs[:72, 192 : 192 + NL] = wc[:, np.asarray(ORDER[128:])].T
    bcv = np.zeros((128, 1), np.float32)
    bcv[:NL, 0] = bc
    bcv[64 : 64 + NL, 0] = bc

    w2q = w2q.astype(ml_dtypes.bfloat16)
    w3q = w3q.astype(ml_dtypes.bfloat16)
    w4q = w4q.astype(ml_dtypes.bfloat16)
    wcs = wcs.astype(ml_dtypes.bfloat16)
    shared = {
        "w1s": w1s, "w2q": w2q, "w3q": w3q, "w4q": w4q, "wcs": wcs,
        "b1v": b1v, "b2v": b2v, "b3v": b3v, "b4v": b4v, "bcv": bcv,
    }
    in_maps = []
    for c in range(NCORES):
        m = dict(shared)
        xc = np.ascontiguousarray(
            x[BC * c : BC * (c + 1)].T).astype(ml_dtypes.float8_e4m3)  # [G, BC]
        # host-side gather into SBUF tile layout: xg[sg][k, l*BC+c] =
        # x^T[gene k of pathway 12*sg+l, c]
        xg = np.zeros((NSG, KPAD, SGS * BC), ml_dtypes.float8_e4m3)
        for sg in range(NSG):
            sel = idx_mat[SGS * sg : SGS * sg + SGS]     # [npth, KPAD]
            npth = sel.shape[0]
            blk = xc[sel]                                # [npth, KPAD, BC]
            xg[sg, :, : npth * BC] = blk.transpose(1, 0, 2).reshape(KPAD, -1)
        m["xg"] = xg
        in_maps.append(m)
    return in_maps


def _build(repeat=None):
    """Build + compile the per-core Bass program (shared across cores).

    repeat: if set, wrap the whole compute body in an on-device For_i loop
    (used only for timing measurements; outputs are identical)."""
    nc = bacc.Bacc("TRN2", target_bir_lowering=False, debug=False,
                   enable_asserts=False)

    dram_in = {}
    for name, shape, dt_ in [
        ("xg", [NSG, KPAD, SGS * BC], F8), ("w1s", [KPAD, 32 * NPATH], F16),
        ("w2q", [128, 64 * NQUAD], BF16), ("w3q", [128, 64 * NH2T], BF16),
        ("w4q", [128, 32 * NH3T], BF16), ("wcs", [128, 256], BF16),
        ("b1v", [128, NQUAD], F32), ("b2v", [128, NH2T], F32),
        ("b3v", [128, NH3T], F32), ("b4v", [128, 2], F32),
        ("bcv", [128, 1], F32),
    ]:
        dram_in[name] = nc.dram_tensor(name, shape, dt_, kind="ExternalInput").ap()
    # out[half] = labels x 512 columns; host transposes/concats
    out_d = nc.dram_tensor("out", [2, NL, NB], F32, kind="ExternalOutput").ap()

    with tile.TileContext(nc) as tc:
        const = tc.alloc_tile_pool(name="const", bufs=1, space="SBUF")
        cs = {}
        for name, ap in dram_in.items():
            if name == "xg":
                continue  # streamed / resident per supergroup
            t = const.tile(ap.shape, ap.dtype, name=f"c_{name}")
            nc.sync.dma_start(t[:], ap[:])
            cs[name] = t

        # resident supergroups: loaded once, reused every iteration
        res_gt = {}
        for sg in range(NRES):
            npth_r = len(_sg_paths(sg))
            rt = const.tile([KPAD, npth_r, BC], F8, name=f"c_xg{sg}")
            nc.sync.dma_start(
                rt[:],
                dram_in["xg"][sg][:, : npth_r * BC].rearrange(
                    "k (l c) -> k l c", l=npth_r),
            )
            res_gt[sg] = rt

        gpool = tc.alloc_tile_pool(name="gath", bufs=3, space="SBUF")
        h1p = tc.alloc_tile_pool(name="h1", bufs=10, space="SBUF")
        h2p = tc.alloc_tile_pool(name="h2", bufs=8, space="SBUF")
        h3p = tc.alloc_tile_pool(name="h3", bufs=6, space="SBUF")
        scp = tc.alloc_tile_pool(name="sc", bufs=5, space="SBUF")
        otp = tc.alloc_tile_pool(name="ot", bufs=2, space="SBUF")
        ps1 = tc.alloc_tile_pool(name="ps1", bufs=2, space="PSUM")
        ps2 = tc.alloc_tile_pool(name="ps2", bufs=2, space="PSUM")
        ps3 = tc.alloc_tile_pool(name="ps3", bufs=2, space="PSUM")
        ps4 = tc.alloc_tile_pool(name="ps4", bufs=2, space="PSUM")

        sg_order = _sg_order()
        # activation engine balancer: only ACT and DVE can read PSUM
        # (GPSIMD has no PSUM route); costs ~ns per [*,512] op
        eng_cost = {"s": 602.0, "v": 703.0}

        loop = tc.For_i(0, repeat, 1) if repeat else contextlib.nullcontext()
        with loop:
            eng_load = {"s": 0.0, "v": 0.0}

            def act(out_ap, in_ap, bias_ap):
                e = min(eng_load, key=lambda k: eng_load[k] + eng_cost[k])
                eng_load[e] += eng_cost[e]
                if e == "s":
                    nc.scalar.activation(out_ap, in_ap, RELU, bias=bias_ap)
                else:
                    nc.vector.tensor_scalar(out_ap, in_ap, bias_ap, 0.0,
                                            ADD, MAX)

            h1t = {}   # (i, half) -> sbuf tile (processing quad index i)
            p2 = {}    # (m, half) -> psum tile
            h2t = {}   # (m, half)
            p3 = {}    # (n, half)
            h3t = {}   # (n, half)
            p4 = {}    # (T, mp, half)
            sct = {}   # (T, half)
            state = {"quads": 0, "l2": 0, "l3": 0, "l4": 0}

            def emit_l2_pair(m):
                # quads 2m (strips {0,1}) and 2m+1 ({2,3}); emission order
                # interleaves sides so consecutive matmuls hit disjoint
                # strips and run concurrently on the PE
                for half in range(2):
                    p2[(m, half)] = ps2.tile([128, NB], F32,
                                             name="p2", tag="p2")
                for half in range(2):
                    for side in range(2):
                        i = 2 * m + side
                        nc.tensor.matmul(
                            p2[(m, half)][64 * side : 64 * side + 64, :],
                            cs["w2q"][:, 64 * i : 64 * i + 64],
                            h1t[(i, half)][:],
                            start=True, stop=True,
                            tile_position=(0, 64 * side),
                        )
                for half in range(2):
                    h2 = h2p.tile([128, NB], BF16, name="h2t", tag="h2t")
                    act(h2[:], p2[(m, half)][:], cs["b2v"][:, m : m + 1])
                    h2t[(m, half)] = h2
                    del p2[(m, half)]
                for side in range(2):
                    del h1t[(2 * m + side, 0)], h1t[(2 * m + side, 1)]

            def emit_l3_pair(n):
                nm = 2 if n < NH3T - 1 else 1   # h2 tiles in this pair
                rows = 128 if nm == 2 else 64
                for half in range(2):
                    p3[(n, half)] = ps3.tile([128, NB], F32,
                                             name="p3", tag="p3")
                for half in range(2):
                    for side in range(nm):
                        m = 2 * n + side
                        nc.tensor.matmul(
                            p3[(n, half)][64 * side : 64 * side + 64, :],
                            cs["w3q"][:, 64 * m : 64 * m + 64],
                            h2t[(m, half)][:],
                            start=True, stop=True,
                            tile_position=(0, 64 * side),
                        )
                for half in range(2):
                    h3 = h3p.tile([128, NB], BF16, name="h3t", tag="h3t")
                    act(h3[:rows, :], p3[(n, half)][:rows, :],
                        cs["b3v"][:rows, n : n + 1])
                    h3t[(n, half)] = h3
                    del p3[(n, half)]
                for side in range(nm):
                    del h2t[(2 * n + side, 0)], h2t[(2 * n + side, 1)]

            def emit_l4(n):
                # sc psum per (T, half) is long-lived: regions rows 32*mm
                # are written by h3-tile pairs (start on even n per region),
                # one act evicts the whole tile at the end.
                T = 0 if n < 8 else 1
                mp = (n - 8 * T) // 2
                mm = mp % 4
                first_region = (n % 2 == 0)
                close_tile = (n == 7) or (n == NH3T - 1)
                srows = 128 if T == 0 else 80
                rows = 128 if n < NH3T - 1 else 64
                for half in range(2):
                    if n % 8 == 0:
                        p4[(T, half)] = ps4.tile([128, NB], F32,
                                                 name="p4", tag="p4")
                    nc.tensor.matmul(
                        p4[(T, half)][32 * mm : 32 * mm + 32, :],
                        cs["w4q"][:rows, 32 * n : 32 * n + 32],
                        h3t[(n, half)][:rows, :],
                        start=first_region,
                        stop=(not first_region) or n == NH3T - 1,
                        tile_position=(0, 32 * mm),
                    )
                    if close_tile:
                        sc = scp.tile([128, NB], BF16, name="sct", tag="sct")
                        act(sc[:srows, :], p4[(T, half)][:srows, :],
                            cs["b4v"][:srows, T : T + 1])
                        sct[(T, half)] = sc
                        del p4[(T, half)]
                for half in range(2):
                    del h3t[(n, half)]

            def drain(flush=False):
                # state counts: l2 = h2 tiles (pairs of quads) emitted,
                # l3 = h3 tiles (pairs of h2 tiles) emitted, l4 = h3 tiles
                # pushed through L4. Each stage lags its producer unless
                # flushing, so PE instructions never wait on a fresh act.
                lag = 0 if flush else 1
                while state["l2"] < (state["quads"] - (0 if flush else 2)) // 2:
                    emit_l2_pair(state["l2"])
                    state["l2"] += 1
                avail3 = state["l2"] // 2 + (1 if state["l2"] == NH2T else 0)
                while state["l3"] < avail3 - lag:
                    emit_l3_pair(state["l3"])
                    state["l3"] += 1
                while state["l4"] < state["l3"] - lag:
                    emit_l4(state["l4"])
                    state["l4"] += 1

            for sg in sg_order:
                npth = len(_sg_paths(sg))
                nq = (npth + 3) // 4
                if sg in res_gt:
                    gt = res_gt[sg]
                else:
                    gt = gpool.tile([KPAD, npth, BC], F8, name="gt", tag="gt")
                    # alternate the two HWDGE rings so per-transfer
                    # doorbell/completion gaps overlap across streams
                    eng = nc.sync if sg % 2 == 0 else nc.scalar
                    eng.dma_start(
                        gt[:],
                        dram_in["xg"][sg][:, : npth * BC].rearrange(
                            "k (l c) -> k l c", l=npth),
                    )
                # ---- L1 ----
                for g in range(nq):
                    i = state["quads"]  # processing quad index
                    for half in range(2):
                        cl = slice(half * NB, half * NB + NB)
                        p1 = ps1.tile([128, NB], F32, name="p1", tag="p1")
                        for j in range(4):
                            p = SGS * sg + 4 * g + j    # global path
                            nc.tensor.matmul(
                                p1[32 * j : 32 * j + 32, :],
                                cs["w1s"][:, 32 * p : 32 * p + 32],
                                gt[:, 4 * g + j, cl],
                                start=True, stop=True,
                                tile_position=(0, 32 * j),
                            )
                        h1 = h1p.tile([128, NB], BF16, name="h1t", tag="h1t")
                        act(h1[:], p1[:], cs["b1v"][:, i : i + 1])
                        h1t[(i, half)] = h1
                    state["quads"] += 1
                    drain()
            drain(flush=True)

            # ---- classifier ----
            pc = ps2.tile([128, NB], F32, name="pc", tag="p2")
            for half in range(2):
                nc.tensor.matmul(
                    pc[64 * half : 64 * half + 64, :],
                    cs["wcs"][:, 64 * half : 64 * half + 64],
                    sct[(0, half)][:],
                    start=True, stop=False,
                    tile_position=(0, 64 * half),
                )
                nc.tensor.matmul(
                    pc[64 * half : 64 * half + 64, :],
                    cs["wcs"][:72, 128 + 64 * half : 192 + 64 * half],
                    sct[(1, half)][:72, :],
                    start=False, stop=True,
                    tile_position=(0, 64 * half),
                )
            sct.clear()
            ot = otp.tile([128, NB], F32, name="ott", tag="ott")
            nc.scalar.activation(ot[:], pc[:], IDENT, bias=cs["bcv"][:, 0:1])
            nc.sync.dma_start(out_d[0], ot[:NL, :])
            nc.sync.dma_start(out_d[1], ot[64 : 64 + NL, :])

        for pl in (ps4, ps3, ps2, ps1, otp, scp,
                   h3p, h2p, h1p, gpool, const):
            pl.release()

    nc.compile()
    return nc


def get_compiled():
    global _COMPILED
    if _COMPILED is None:
        _COMPILED = _build()
    return _COMPILED


def kernel(**inputs):
    nc = get_compiled()
    in_maps = _pack(inputs)
    res = run_bass_kernel_spmd(nc, in_maps, core_ids=list(range(NCORES)))
    outs = []
    for c in range(NCORES):
        o = res.results[c]["out"]  # [2, NL, NB]
        outs.append(o[0].T)
        outs.append(o[1].T)
    return np.ascontiguousarray(np.concatenate(outs, axis=0))


if __name__ == "__main__":
    print("built", get_compiled())
